# revision 18
# baseline (speedup 1.0000x reference)
"""Trainium2 Bass kernel for a binarized-conv BasicBlock (2x BinConv3x3 + BN + residual + PReLU).

Strategy (8 NeuronCores, data-parallel over batch):
  - 64 images -> 8 per core; binarized conv weights / BN / PReLU params replicated.
  - Binarized values are exactly +/-1, so fp8 matmuls are numerically exact
    (fp32 PSUM accumulation of small integers). perf_mode=DoubleRow packs the
    full 256-channel contraction into one matmul per 3x3 tap.
  - Conv3x3 as implicit GEMM: per output tile [128 Cout x 392 cols] accumulate
    9 tap matmuls reading shifted windows of a zero-padded (30x30) binarized
    activation image.
  - BatchNorm uses full-batch statistics: per-channel sum/sumsq partials from
    bn_stats fused with PSUM evacuation, one tiny (2KB) AllGather per BN plus
    an on-device 8-way sum; a warm-up collective hides first-collective cost.
  - PReLU runs on the scalar engine's Prelu activation with runtime alpha.
"""

import numpy as np
import ml_dtypes

import concourse.bacc as bacc
import concourse.mybir as mybir
import concourse.tile as tile
from concourse.tile_rust import add_dep_helper
from concourse import bass_utils

N_CORES = 8
B_FULL, C, H, W = 64, 256, 28, 28
BL = B_FULL // N_CORES  # images per core
P = 128
NB = C // P             # channel blocks
HW = H * W              # 784
PADL = 30               # padded row length
PADQ = PADL * PADL      # 900 padded image
HALF = 14 * W           # 392 columns per psum tile (half an image)
NI_LOCAL = float(BL * HW)      # interior elems per core per channel
N_TOT = float(B_FULL * HW)     # full-batch elems per channel
SCALE = 0.1
BN_EPS = 1e-5

F32 = mybir.dt.float32
BF16 = mybir.dt.bfloat16
FP8 = mybir.dt.float8e4
BF16_NP = np.dtype(ml_dtypes.bfloat16)

_CACHE: dict = {}


def _build():
    nc = bacc.Bacc("TRN2", target_bir_lowering=False, debug=False,
                   num_devices=N_CORES)
    F = mybir.ActivationFunctionType
    Op = mybir.AluOpType
    DR = mybir.MatmulPerfMode.DoubleRow

    x_d = nc.dram_tensor("x", [BL, C, H, W], F32, kind="ExternalInput")
    xh_d = nc.dram_tensor("xh", [BL, C, H, W], BF16, kind="ExternalInput")
    # weights packed [ki, tap, i, mblk, co] with channel c = i*128 + ki
    w1_d = nc.dram_tensor("w1", [P, 9, NB, NB, P], FP8, kind="ExternalInput")
    w2_d = nc.dram_tensor("w2", [P, 9, NB, NB, P], FP8, kind="ExternalInput")
    # params packed [P, 6, NB]: order (s1,g1,be1,s2,g2,be2)
    par_d = nc.dram_tensor("par", [P, 6, NB], F32, kind="ExternalInput")
    a1_d = nc.dram_tensor("a1", [1], F32, kind="ExternalInput")
    a2_d = nc.dram_tensor("a2", [1], F32, kind="ExternalInput")
    o_d = nc.dram_tensor("o", [BL, C, H, W], F32, kind="ExternalOutput")

    with tile.TileContext(nc) as tc:
        with (
            tc.tile_pool(name="sbuf", bufs=1) as sbuf,
            tc.tile_pool(name="psum", bufs=6, space="PSUM") as psum_pool,
            tc.tile_pool(name="dram", bufs=1, space="DRAM") as dram,
        ):
            # ---- static parameters ----
            w1_sb = sbuf.tile([P, 9, NB, NB, P], FP8)
            w2_sb = sbuf.tile([P, 9, NB, NB, P], FP8)
            nc.sync.dma_start(w1_sb[:], w1_d[:, :, :, :, :])
            nc.sync.dma_start(w2_sb[:], w2_d[:, :, :, :, :])

            par_sb = sbuf.tile([P, 6, NB], F32)
            nc.sync.dma_start(par_sb[:], par_d[:, :, :])
            s1_sb, g1_sb, be1_sb = (par_sb[:, i, :] for i in range(3))
            s2_sb, g2_sb, be2_sb = (par_sb[:, i, :] for i in range(3, 6))
            a1_sb = sbuf.tile([P, 1], F32)
            a2_sb = sbuf.tile([P, 1], F32)
            nc.sync.dma_start(a1_sb[:], a1_d[None, :].partition_broadcast(P))
            nc.sync.dma_start(a2_sb[:], a2_d[None, :].partition_broadcast(P))

            # s^2 and gamma*s (pre-collective, off the critical path)
            sq1 = sbuf.tile([P, NB], F32)
            gs1 = sbuf.tile([P, NB], F32)
            sq2 = sbuf.tile([P, NB], F32)
            gs2 = sbuf.tile([P, NB], F32)
            nc.vector.tensor_tensor(sq1[:], s1_sb, s1_sb, Op.mult)
            nc.vector.tensor_tensor(gs1[:], g1_sb, s1_sb, Op.mult)
            nc.vector.tensor_tensor(sq2[:], s2_sb, s2_sb, Op.mult)
            nc.vector.tensor_tensor(gs2[:], g2_sb, s2_sb, Op.mult)

            # tiny warm-up collective: absorbs the first-collective ncfw
            # wakeup / cross-core skew while conv1 runs
            warm_in = dram.tile([4, 2], F32, name="warm_in")
            warm_out = dram.tile([N_CORES * 4, 2], F32, name="warm_out",
                                 addr_space="Shared")
            nc.sync.dma_start(warm_in[:], par_d[0, 0:4, :])
            nc.gpsimd.collective_compute(
                "AllGather", Op.bypass,
                replica_groups=[list(range(N_CORES))],
                ins=[warm_in[:]], outs=[warm_out[:]])

            # ---- activations ----
            xb_img = [sbuf.tile([P, NB, PADQ], FP8, name=f"xb{b}")
                      for b in range(BL)]
            xh_img = [sbuf.tile([P, NB, HW], BF16, name=f"xh{b}")
                      for b in range(BL)]
            x_img = [sbuf.tile([P, NB, HW], F32, name=f"xr{b}")
                     for b in range(BL)]
            y_img = [sbuf.tile([P, NB, HW], F32, name=f"yy{b}")
                     for b in range(BL)]
            xbv = [t.rearrange("p k (r c) -> p k r c", c=PADL) for t in xb_img]
            xhv = [t.rearrange("p k (r c) -> p k r c", c=W) for t in xh_img]
            xv = [t.rearrange("p k (r c) -> p k r c", c=W) for t in x_img]
            yv = [t.rearrange("p k (r c) -> p k r c", c=W) for t in y_img]

            # bf16 copy of x loads fast (half the bytes, halves per-image
            # latency) and sign(bf16(x)) == sign(x); the f32 copy for the
            # residual streams in lazily under conv1
            xh_src = xh_d.rearrange("b (k p) h w -> b p k (h w)", p=P)
            last_xh_dma = None
            for b in range(BL):
                # zero only the pad borders; sign() fills the interior
                nc.vector.memset(xbv[b][:, :, 0:30:29, :], 0.0)
                nc.vector.memset(xbv[b][:, :, 1:29, 0:30:29], 0.0)
                for k in range(NB):
                    last_xh_dma = nc.sync.dma_start(xh_img[b][:, k, :],
                                                    xh_src[b, :, k, :])
                    nc.scalar.sign(xbv[b][:, k, 1:29, 1:29], xhv[b][:, k])
            # f32 x (residual) streams in only after the bf16 copy is done so
            # it doesn't steal DMA bandwidth from the conv1 critical path
            x_src = x_d.rearrange("b (k p) h w -> b p k (h w)", p=P)
            for b in range(BL):
                dma = nc.gpsimd.dma_start(x_img[b][:], x_src[b])
                add_dep_helper(dma.ins, last_xh_dma.ins, sync=True,
                               reason="f32 x load after bf16 x load")

            def conv(w_sb, y_out, st6):
                """bin-conv3x3 via DoubleRow fp8 (full 256-ch contraction per
                tap); writes raw integer conv sums + per-tile stats."""
                for b in range(BL):
                    for m in range(NB):
                        for hh in range(2):
                            ps = psum_pool.tile([P, HALF], F32, name="ps",
                                                tag="ps")
                            for t in range(9):
                                dh, dw = t // 3, t % 3
                                rhs = xbv[b][:, :,
                                             hh * 14 + dh:hh * 14 + dh + 14,
                                             dw:dw + 28]
                                nc.tensor.matmul(
                                    ps[:], w_sb[:, t, :, m, :], rhs,
                                    start=(t == 0), stop=(t == 8),
                                    perf_mode=DR)
                            dst = y_out[b][:, m, hh * HALF:(hh + 1) * HALF]
                            nc.scalar.copy(dst, ps[:])
                            idx = (b * 2 + hh) * 6
                            nc.vector.bn_stats(st6[:, m, idx:idx + 6], dst)

            def stats_to_ab(st6, sq_sb, gs_sb, be_sb, tagn):
                """aggregate local stats, AllGather per-channel [sum, sumsq]
                partials, 8-way sum, produce per-channel A,B."""
                st2 = sbuf.tile([P, NB, 2], F32, name=f"st2_{tagn}")
                for m in range(NB):
                    nc.vector.bn_aggr(st2[:, m], st6[:, m])
                cc_in = sbuf.tile([P, 4], F32, name=f"ccin_{tagn}")
                tmp = sbuf.tile([P, NB], F32, name=f"ctmp_{tagn}")
                # cols 0:2 = sum(y) per channel; 2:4 = sum(y^2)
                nc.vector.tensor_scalar(cc_in[:, 0:2], st2[:, :, 0], NI_LOCAL,
                                        None, Op.mult)
                nc.vector.tensor_tensor(tmp[:], st2[:, :, 0], st2[:, :, 0],
                                        Op.mult)
                nc.vector.tensor_tensor(tmp[:], st2[:, :, 1], tmp[:], Op.add)
                nc.vector.tensor_scalar(cc_in[:, 2:4], tmp[:], NI_LOCAL,
                                        None, Op.mult)
                cc_din = dram.tile([P, 4], F32, name=f"ccdin_{tagn}")
                cc_dout = dram.tile([N_CORES, P, 4], F32,
                                    name=f"ccdout_{tagn}",
                                    addr_space="Shared")
                nc.sync.dma_start(cc_din[:], cc_in[:])
                nc.gpsimd.collective_compute(
                    "AllGather", Op.bypass,
                    replica_groups=[list(range(N_CORES))],
                    ins=[cc_din[:]], outs=[cc_dout[:]])
                ccg = sbuf.tile([P, N_CORES, 4], F32, name=f"ccg_{tagn}")
                nc.sync.dma_start(ccg[:], cc_dout.rearrange("r p j -> p r j"))
                # 8-way tree sum -> [P, 4]
                nc.vector.tensor_tensor(ccg[:, 0:4], ccg[:, 0:4], ccg[:, 4:8],
                                        Op.add)
                nc.vector.tensor_tensor(ccg[:, 0:2], ccg[:, 0:2], ccg[:, 2:4],
                                        Op.add)
                cc_out = sbuf.tile([P, 4], F32, name=f"ccout_{tagn}")
                nc.vector.tensor_tensor(cc_out[:], ccg[:, 0], ccg[:, 1],
                                        Op.add)

                # [mean_m0, mean_m1, E[y^2]_m0, E[y^2]_m1] in one op
                mq = sbuf.tile([P, 4], F32, name=f"mq_{tagn}")
                nc.vector.tensor_scalar(mq[:], cc_out[:], 1.0 / N_TOT,
                                        None, Op.mult)
                mg = mq[:, 0:2]
                vg = sbuf.tile([P, NB], F32, name=f"vg_{tagn}")
                t0 = sbuf.tile([P, NB], F32, name=f"t0_{tagn}")
                d = sbuf.tile([P, NB], F32, name=f"d_{tagn}")
                r = sbuf.tile([P, NB], F32, name=f"r_{tagn}")
                av = sbuf.tile([P, NB], F32, name=f"av_{tagn}")
                bv = sbuf.tile([P, NB], F32, name=f"bv_{tagn}")
                nc.vector.tensor_tensor(t0[:], mg, mg, Op.mult)
                nc.vector.tensor_tensor(vg[:], mq[:, 2:4], t0[:], Op.subtract)
                # d = s^2 * var_y + eps   (== var(out) + eps up to rounding)
                nc.vector.tensor_tensor(d[:], sq_sb[:], vg[:], Op.mult)
                nc.vector.tensor_scalar(d[:], d[:], BN_EPS, None, Op.add)
                # r = rsqrt(d): sqrt+divide, then one Newton step
                nc.scalar.sqrt(t0[:], d[:])
                nc.vector.reciprocal(r[:], t0[:])
                nc.vector.tensor_tensor(t0[:], r[:], r[:], Op.mult)
                nc.vector.tensor_tensor(t0[:], t0[:], d[:], Op.mult)
                nc.vector.tensor_scalar(t0[:], t0[:], -0.5, 1.5, Op.mult,
                                        Op.add)
                nc.vector.tensor_tensor(r[:], r[:], t0[:], Op.mult)
                # A = gamma * s * r ; B = beta - mean_y * A
                nc.vector.tensor_tensor(av[:], gs_sb[:], r[:], Op.mult)
                nc.vector.tensor_tensor(t0[:], mg[:], av[:], Op.mult)
                nc.vector.tensor_tensor(bv[:], be_sb[:], t0[:], Op.subtract)
                return av, bv

            def post(src_imgs, res_imgs, av, bv, a_sb, write_xb2,
                     tt_gps=()):
                """per-image BN affine + residual + PReLU, split across DVE
                (affine m0, residual add) and ACT (affine m1, prelu).
                Images in tt_gps do the residual add on GpSimd instead."""
                for b in range(BL):
                    u0 = src_imgs[b][:, 0, :]
                    u1 = src_imgs[b][:, 1, :]
                    up = src_imgs[b][:, :, :]
                    nc.vector.tensor_scalar(u0, u0, av[:, 0:1], bv[:, 0:1],
                                            Op.mult, Op.add)
                    nc.scalar.activation(u1, u1, F.Identity,
                                         bias=bv[:, 1:2], scale=av[:, 1:2])
                    tt_eng = nc.gpsimd if b in tt_gps else nc.vector
                    tt_eng.tensor_tensor(up, up, res_imgs[b][:, :, :],
                                         Op.add)
                    nc.scalar.activation(
                        src_imgs[b].rearrange("p k i -> p (k i)"),
                        src_imgs[b].rearrange("p k i -> p (k i)"),
                        F.Prelu, bias=0.0, scale=1.0, alpha=a_sb[:, 0:1])
                    if write_xb2:
                        sv = src_imgs[b].rearrange("p k (r c) -> p k r c", c=W)
                        nc.scalar.sign(xbv[b][:, :, 1:29, 1:29], sv[:, :, :, :])

            # ================= stage 1 =================
            st6_1 = sbuf.tile([P, NB, BL * 12], F32)
            conv(w1_sb, y_img, st6_1)
            a1v, b1v = stats_to_ab(st6_1, sq1, gs1, be1_sb, "c1")
            post(y_img, x_img, a1v, b1v, a1_sb, write_xb2=True)

            # ================= stage 2 =================
            st6_2 = sbuf.tile([P, NB, BL * 12], F32)
            conv(w2_sb, x_img, st6_2)  # y2 overwrites x
            a2v, b2v = stats_to_ab(st6_2, sq2, gs2, be2_sb, "c2")
            post(x_img, y_img, a2v, b2v, a2_sb, write_xb2=False,
                 tt_gps=(5, 6, 7))

            o_dst = o_d.rearrange("b (k p) h w -> b p k (h w)", p=P)
            for b in range(BL):
                nc.gpsimd.dma_start(o_dst[b], x_img[b][:])

    nc.compile()
    return nc


def _get_nc():
    if "nc" not in _CACHE:
        _CACHE["nc"] = _build()
    return _CACHE["nc"]


def _pack_w(w):
    wb = np.sign(np.asarray(w, np.float32))
    # [co, ci, kh, kw] -> [ki, tap, i, co_blk, co] with ci = i*128 + ki
    t = wb.reshape(NB, P, NB, P, 3, 3)
    t = np.transpose(t, (3, 4, 5, 2, 0, 1)).reshape(P, 9, NB, NB, P)
    return np.ascontiguousarray(t).astype(np.dtype(ml_dtypes.float8_e4m3))


def _pack_par(s1, g1, be1, s2, g2, be2):
    par = np.stack([np.asarray(v, np.float32).reshape(NB, P)
                    for v in (s1, g1, be1, s2, g2, be2)])  # [6, NB, P]
    return np.ascontiguousarray(par.transpose(2, 0, 1))    # [P, 6, NB]


def kernel(x, conv1_w, conv2_w, bn1_gamma, bn1_beta, bn2_gamma, bn2_beta,
           prelu1_a, prelu2_a):
    x = np.ascontiguousarray(np.asarray(x, np.float32))
    nc = _get_nc()

    s1 = SCALE * np.mean(np.abs(np.asarray(conv1_w, np.float32)),
                         axis=(1, 2, 3), dtype=np.float32)
    s2 = SCALE * np.mean(np.abs(np.asarray(conv2_w, np.float32)),
                         axis=(1, 2, 3), dtype=np.float32)

    shared = {
        "w1": _pack_w(conv1_w), "w2": _pack_w(conv2_w),
        "par": _pack_par(s1, bn1_gamma, bn1_beta, s2, bn2_gamma, bn2_beta),
        "a1": np.asarray(prelu1_a, np.float32).reshape(1),
        "a2": np.asarray(prelu2_a, np.float32).reshape(1),
    }
    xh = x.astype(BF16_NP)
    in_maps = [dict(shared, x=x[c * BL:(c + 1) * BL],
                    xh=xh[c * BL:(c + 1) * BL]) for c in range(N_CORES)]

    res = bass_utils.run_bass_kernel_spmd(nc, in_maps,
                                          core_ids=list(range(N_CORES)))
    out = np.concatenate([res.results[c]["o"] for c in range(N_CORES)], axis=0)
    return out


# revision 22
# speedup vs baseline: 1.0347x; 1.0347x over previous
"""Trainium2 Bass kernel for a binarized-conv BasicBlock (2x BinConv3x3 + BN + residual + PReLU).

Strategy (8 NeuronCores, data-parallel over batch):
  - 64 images -> 8 per core; binarized conv weights / BN / PReLU params replicated.
  - Binarized values are exactly +/-1, so fp8 matmuls are numerically exact
    (fp32 PSUM accumulation of small integers). perf_mode=DoubleRow packs the
    full 256-channel contraction into one matmul per 3x3 tap.
  - Conv3x3 as implicit GEMM: per output tile [128 Cout x 392 cols] accumulate
    9 tap matmuls reading shifted windows of a zero-padded (30x30) binarized
    activation image.
  - BatchNorm uses full-batch statistics: per-channel sum/sumsq partials from
    bn_stats fused with PSUM evacuation, one tiny (2KB) AllGather per BN plus
    an on-device 8-way sum; a warm-up collective hides first-collective cost.
  - PReLU runs on the scalar engine's Prelu activation with runtime alpha.
"""

import numpy as np
import ml_dtypes

import concourse.bacc as bacc
import concourse.mybir as mybir
import concourse.tile as tile
from concourse.tile_rust import add_dep_helper
from concourse import bass_utils

N_CORES = 8
B_FULL, C, H, W = 64, 256, 28, 28
BL = B_FULL // N_CORES  # images per core
P = 128
NB = C // P             # channel blocks
HW = H * W              # 784
PADL = 30               # padded row length
PADQ = PADL * PADL      # 900 padded image
HALF = 14 * W           # 392 columns per psum tile (half an image)
NI_LOCAL = float(BL * HW)      # interior elems per core per channel
N_TOT = float(B_FULL * HW)     # full-batch elems per channel
SCALE = 0.1
BN_EPS = 1e-5

F32 = mybir.dt.float32
BF16 = mybir.dt.bfloat16
FP8 = mybir.dt.float8e4
BF16_NP = np.dtype(ml_dtypes.bfloat16)

_CACHE: dict = {}


def _build():
    nc = bacc.Bacc("TRN2", target_bir_lowering=False, debug=False,
                   num_devices=N_CORES)
    F = mybir.ActivationFunctionType
    Op = mybir.AluOpType
    DR = mybir.MatmulPerfMode.DoubleRow

    x_d = nc.dram_tensor("x", [BL, C, H, W], F32, kind="ExternalInput")
    xh_d = nc.dram_tensor("xh", [BL, C, H, W], BF16, kind="ExternalInput")
    # weights packed [ki, tap, i, mblk, co] with channel c = i*128 + ki
    w1_d = nc.dram_tensor("w1", [P, 9, NB, NB, P], FP8, kind="ExternalInput")
    w2_d = nc.dram_tensor("w2", [P, 9, NB, NB, P], FP8, kind="ExternalInput")
    # params packed [P, 6, NB]: order (s1,g1,be1,s2,g2,be2)
    par_d = nc.dram_tensor("par", [P, 6, NB], F32, kind="ExternalInput")
    a1_d = nc.dram_tensor("a1", [1], F32, kind="ExternalInput")
    a2_d = nc.dram_tensor("a2", [1], F32, kind="ExternalInput")
    o_d = nc.dram_tensor("o", [BL, C, H, W], F32, kind="ExternalOutput")

    with tile.TileContext(nc) as tc:
        with (
            tc.tile_pool(name="sbuf", bufs=1) as sbuf,
            tc.tile_pool(name="psum", bufs=6, space="PSUM") as psum_pool,
            tc.tile_pool(name="dram", bufs=1, space="DRAM") as dram,
        ):
            # ---- static parameters ----
            # conv1 weights load first: they gate the first matmul; conv2
            # weights aren't needed until halfway through the kernel
            w1_sb = sbuf.tile([P, 9, NB, NB, P], FP8)
            w2_sb = sbuf.tile([P, 9, NB, NB, P], FP8)
            nc.sync.dma_start(w1_sb[:], w1_d[:, :, :, :, :])

            par_sb = sbuf.tile([P, 6, NB], F32)
            nc.sync.dma_start(par_sb[:], par_d[:, :, :])
            s1_sb, g1_sb, be1_sb = (par_sb[:, i, :] for i in range(3))
            s2_sb, g2_sb, be2_sb = (par_sb[:, i, :] for i in range(3, 6))
            a1_sb = sbuf.tile([P, 1], F32)
            a2_sb = sbuf.tile([P, 1], F32)
            nc.sync.dma_start(a1_sb[:], a1_d[None, :].partition_broadcast(P))
            nc.sync.dma_start(a2_sb[:], a2_d[None, :].partition_broadcast(P))

            # s^2 and gamma*s (pre-collective, off the critical path)
            sq1 = sbuf.tile([P, NB], F32)
            gs1 = sbuf.tile([P, NB], F32)
            sq2 = sbuf.tile([P, NB], F32)
            gs2 = sbuf.tile([P, NB], F32)
            nc.vector.tensor_tensor(sq1[:], s1_sb, s1_sb, Op.mult)
            nc.vector.tensor_tensor(gs1[:], g1_sb, s1_sb, Op.mult)
            nc.vector.tensor_tensor(sq2[:], s2_sb, s2_sb, Op.mult)
            nc.vector.tensor_tensor(gs2[:], g2_sb, s2_sb, Op.mult)

            # tiny warm-up collective: absorbs the first-collective ncfw
            # wakeup / cross-core skew while conv1 runs
            warm_in = dram.tile([4, 2], F32, name="warm_in")
            warm_out = dram.tile([N_CORES * 4, 2], F32, name="warm_out",
                                 addr_space="Shared")
            nc.sync.dma_start(warm_in[:], par_d[0, 0:4, :])
            nc.gpsimd.collective_compute(
                "AllGather", Op.bypass,
                replica_groups=[list(range(N_CORES))],
                ins=[warm_in[:]], outs=[warm_out[:]])

            # ---- activations ----
            xb_img = [sbuf.tile([P, NB, PADQ], FP8, name=f"xb{b}")
                      for b in range(BL)]
            xh_img = [sbuf.tile([P, NB, HW], BF16, name=f"xh{b}")
                      for b in range(BL)]
            x_img = [sbuf.tile([P, NB, HW], F32, name=f"xr{b}")
                     for b in range(BL)]
            y_img = [sbuf.tile([P, NB, HW], F32, name=f"yy{b}")
                     for b in range(BL)]
            xbv = [t.rearrange("p k (r c) -> p k r c", c=PADL) for t in xb_img]
            xhv = [t.rearrange("p k (r c) -> p k r c", c=W) for t in xh_img]
            xv = [t.rearrange("p k (r c) -> p k r c", c=W) for t in x_img]
            yv = [t.rearrange("p k (r c) -> p k r c", c=W) for t in y_img]

            # bf16 copy of x loads fast (half the bytes, halves per-image
            # latency) and sign(bf16(x)) == sign(x); the f32 copy for the
            # residual streams in lazily under conv1
            xh_src = xh_d.rearrange("b (k p) h w -> b p k (h w)", p=P)
            last_xh_dma = None
            for b in range(BL):
                # zero only the pad borders; sign() fills the interior
                nc.vector.memset(xbv[b][:, :, 0:30:29, :], 0.0)
                nc.vector.memset(xbv[b][:, :, 1:29, 0:30:29], 0.0)
                for k in range(NB):
                    last_xh_dma = nc.sync.dma_start(xh_img[b][:, k, :],
                                                    xh_src[b, :, k, :])
                    nc.scalar.sign(xbv[b][:, k, 1:29, 1:29], xhv[b][:, k])
            # conv2 weights + f32 x (residual) stream in only after the bf16
            # copy is done so they don't steal DMA bandwidth from the conv1
            # critical path
            w2dma = nc.gpsimd.dma_start(w2_sb[:], w2_d[:, :, :, :, :])
            add_dep_helper(w2dma.ins, last_xh_dma.ins, sync=True,
                           reason="w2 load after bf16 x load")
            x_src = x_d.rearrange("b (k p) h w -> b p k (h w)", p=P)
            for b in range(BL):
                dma = nc.gpsimd.dma_start(x_img[b][:], x_src[b])
                add_dep_helper(dma.ins, last_xh_dma.ins, sync=True,
                               reason="f32 x load after bf16 x load")

            def conv(w_sb, y_out, st6):
                """bin-conv3x3 via DoubleRow fp8 (full 256-ch contraction per
                tap); writes raw integer conv sums + per-tile stats."""
                for b in range(BL):
                    for m in range(NB):
                        for hh in range(2):
                            ps = psum_pool.tile([P, HALF], F32, name="ps",
                                                tag="ps")
                            for t in range(9):
                                dh, dw = t // 3, t % 3
                                rhs = xbv[b][:, :,
                                             hh * 14 + dh:hh * 14 + dh + 14,
                                             dw:dw + 28]
                                nc.tensor.matmul(
                                    ps[:], w_sb[:, t, :, m, :], rhs,
                                    start=(t == 0), stop=(t == 8),
                                    perf_mode=DR)
                            dst = y_out[b][:, m, hh * HALF:(hh + 1) * HALF]
                            nc.scalar.copy(dst, ps[:])
                            idx = (b * 2 + hh) * 6
                            nc.vector.bn_stats(st6[:, m, idx:idx + 6], dst)

            def stats_to_ab(st6, sq_sb, gs_sb, be_sb, tagn):
                """aggregate local stats, AllGather per-channel [sum, sumsq]
                partials, 8-way sum, produce per-channel A,B."""
                st2 = sbuf.tile([P, NB, 2], F32, name=f"st2_{tagn}")
                for m in range(NB):
                    nc.vector.bn_aggr(st2[:, m], st6[:, m])
                cc_in = sbuf.tile([P, 4], F32, name=f"ccin_{tagn}")
                tmp = sbuf.tile([P, NB], F32, name=f"ctmp_{tagn}")
                # cols 0:2 = sum(y) per channel; 2:4 = sum(y^2)
                nc.vector.tensor_scalar(cc_in[:, 0:2], st2[:, :, 0], NI_LOCAL,
                                        None, Op.mult)
                nc.vector.tensor_tensor(tmp[:], st2[:, :, 0], st2[:, :, 0],
                                        Op.mult)
                nc.vector.tensor_tensor(tmp[:], st2[:, :, 1], tmp[:], Op.add)
                nc.vector.tensor_scalar(cc_in[:, 2:4], tmp[:], NI_LOCAL,
                                        None, Op.mult)
                cc_din = dram.tile([P, 4], F32, name=f"ccdin_{tagn}")
                cc_dout = dram.tile([N_CORES, P, 4], F32,
                                    name=f"ccdout_{tagn}",
                                    addr_space="Shared")
                nc.sync.dma_start(cc_din[:], cc_in[:])
                nc.gpsimd.collective_compute(
                    "AllGather", Op.bypass,
                    replica_groups=[list(range(N_CORES))],
                    ins=[cc_din[:]], outs=[cc_dout[:]])
                ccg = sbuf.tile([P, N_CORES, 4], F32, name=f"ccg_{tagn}")
                nc.sync.dma_start(ccg[:], cc_dout.rearrange("r p j -> p r j"))
                # 8-way tree sum -> [P, 4]
                nc.vector.tensor_tensor(ccg[:, 0:4], ccg[:, 0:4], ccg[:, 4:8],
                                        Op.add)
                nc.vector.tensor_tensor(ccg[:, 0:2], ccg[:, 0:2], ccg[:, 2:4],
                                        Op.add)
                cc_out = sbuf.tile([P, 4], F32, name=f"ccout_{tagn}")
                nc.vector.tensor_tensor(cc_out[:], ccg[:, 0], ccg[:, 1],
                                        Op.add)

                # [mean_m0, mean_m1, E[y^2]_m0, E[y^2]_m1] in one op
                mq = sbuf.tile([P, 4], F32, name=f"mq_{tagn}")
                nc.vector.tensor_scalar(mq[:], cc_out[:], 1.0 / N_TOT,
                                        None, Op.mult)
                mg = mq[:, 0:2]
                vg = sbuf.tile([P, NB], F32, name=f"vg_{tagn}")
                t0 = sbuf.tile([P, NB], F32, name=f"t0_{tagn}")
                d = sbuf.tile([P, NB], F32, name=f"d_{tagn}")
                r = sbuf.tile([P, NB], F32, name=f"r_{tagn}")
                av = sbuf.tile([P, NB], F32, name=f"av_{tagn}")
                bv = sbuf.tile([P, NB], F32, name=f"bv_{tagn}")
                nc.vector.tensor_tensor(t0[:], mg, mg, Op.mult)
                nc.vector.tensor_tensor(vg[:], mq[:, 2:4], t0[:], Op.subtract)
                # d = s^2 * var_y + eps   (== var(out) + eps up to rounding)
                nc.vector.tensor_tensor(d[:], sq_sb[:], vg[:], Op.mult)
                nc.vector.tensor_scalar(d[:], d[:], BN_EPS, None, Op.add)
                # r = rsqrt(d): sqrt+divide, then one Newton step
                nc.scalar.sqrt(t0[:], d[:])
                nc.vector.reciprocal(r[:], t0[:])
                nc.vector.tensor_tensor(t0[:], r[:], r[:], Op.mult)
                nc.vector.tensor_tensor(t0[:], t0[:], d[:], Op.mult)
                nc.vector.tensor_scalar(t0[:], t0[:], -0.5, 1.5, Op.mult,
                                        Op.add)
                nc.vector.tensor_tensor(r[:], r[:], t0[:], Op.mult)
                # A = gamma * s * r ; B = beta - mean_y * A
                nc.vector.tensor_tensor(av[:], gs_sb[:], r[:], Op.mult)
                nc.vector.tensor_tensor(t0[:], mg[:], av[:], Op.mult)
                nc.vector.tensor_tensor(bv[:], be_sb[:], t0[:], Op.subtract)
                return av, bv

            def post(src_imgs, res_imgs, av, bv, a_sb, write_xb2,
                     tt_gps=()):
                """per-image BN affine (DVE) + residual add (DVE/GpSimd) +
                PReLU (ACT). Images in tt_gps do the residual on GpSimd."""
                for b in range(BL):
                    u0 = src_imgs[b][:, 0, :]
                    u1 = src_imgs[b][:, 1, :]
                    up = src_imgs[b][:, :, :]
                    nc.vector.tensor_scalar(u0, u0, av[:, 0:1], bv[:, 0:1],
                                            Op.mult, Op.add)
                    nc.vector.tensor_scalar(u1, u1, av[:, 1:2], bv[:, 1:2],
                                            Op.mult, Op.add)
                    tt_eng = nc.gpsimd if b in tt_gps else nc.vector
                    tt_eng.tensor_tensor(up, up, res_imgs[b][:, :, :],
                                         Op.add)
                    nc.scalar.activation(
                        src_imgs[b].rearrange("p k i -> p (k i)"),
                        src_imgs[b].rearrange("p k i -> p (k i)"),
                        F.Prelu, bias=0.0, scale=1.0, alpha=a_sb[:, 0:1])
                    if write_xb2:
                        sv = src_imgs[b].rearrange("p k (r c) -> p k r c", c=W)
                        nc.scalar.sign(xbv[b][:, :, 1:29, 1:29], sv[:, :, :, :])

            # ================= stage 1 =================
            st6_1 = sbuf.tile([P, NB, BL * 12], F32)
            conv(w1_sb, y_img, st6_1)
            a1v, b1v = stats_to_ab(st6_1, sq1, gs1, be1_sb, "c1")
            post(y_img, x_img, a1v, b1v, a1_sb, write_xb2=True)

            # ================= stage 2 =================
            st6_2 = sbuf.tile([P, NB, BL * 12], F32)
            conv(w2_sb, x_img, st6_2)  # y2 overwrites x
            a2v, b2v = stats_to_ab(st6_2, sq2, gs2, be2_sb, "c2")
            post(x_img, y_img, a2v, b2v, a2_sb, write_xb2=False,
                 tt_gps=(5, 6, 7))

            o_dst = o_d.rearrange("b (k p) h w -> b p k (h w)", p=P)
            for b in range(BL):
                nc.sync.dma_start(o_dst[b], x_img[b][:])

    nc.compile()
    return nc


def _get_nc():
    if "nc" not in _CACHE:
        _CACHE["nc"] = _build()
    return _CACHE["nc"]


def _pack_w(w):
    wb = np.sign(np.asarray(w, np.float32))
    # [co, ci, kh, kw] -> [ki, tap, i, co_blk, co] with ci = i*128 + ki
    t = wb.reshape(NB, P, NB, P, 3, 3)
    t = np.transpose(t, (3, 4, 5, 2, 0, 1)).reshape(P, 9, NB, NB, P)
    return np.ascontiguousarray(t).astype(np.dtype(ml_dtypes.float8_e4m3))


def _pack_par(s1, g1, be1, s2, g2, be2):
    par = np.stack([np.asarray(v, np.float32).reshape(NB, P)
                    for v in (s1, g1, be1, s2, g2, be2)])  # [6, NB, P]
    return np.ascontiguousarray(par.transpose(2, 0, 1))    # [P, 6, NB]


def kernel(x, conv1_w, conv2_w, bn1_gamma, bn1_beta, bn2_gamma, bn2_beta,
           prelu1_a, prelu2_a):
    x = np.ascontiguousarray(np.asarray(x, np.float32))
    nc = _get_nc()

    s1 = SCALE * np.mean(np.abs(np.asarray(conv1_w, np.float32)),
                         axis=(1, 2, 3), dtype=np.float32)
    s2 = SCALE * np.mean(np.abs(np.asarray(conv2_w, np.float32)),
                         axis=(1, 2, 3), dtype=np.float32)

    shared = {
        "w1": _pack_w(conv1_w), "w2": _pack_w(conv2_w),
        "par": _pack_par(s1, bn1_gamma, bn1_beta, s2, bn2_gamma, bn2_beta),
        "a1": np.asarray(prelu1_a, np.float32).reshape(1),
        "a2": np.asarray(prelu2_a, np.float32).reshape(1),
    }
    xh = x.astype(BF16_NP)
    in_maps = [dict(shared, x=x[c * BL:(c + 1) * BL],
                    xh=xh[c * BL:(c + 1) * BL]) for c in range(N_CORES)]

    res = bass_utils.run_bass_kernel_spmd(nc, in_maps,
                                          core_ids=list(range(N_CORES)))
    out = np.concatenate([res.results[c]["o"] for c in range(N_CORES)], axis=0)
    return out


# revision 23
# speedup vs baseline: 1.0878x; 1.0513x over previous
"""Trainium2 Bass kernel for a binarized-conv BasicBlock (2x BinConv3x3 + BN + residual + PReLU).

Strategy (8 NeuronCores, data-parallel over batch):
  - 64 images -> 8 per core; binarized conv weights / BN / PReLU params replicated.
  - Binarized values are exactly +/-1, so fp8 matmuls are numerically exact
    (fp32 PSUM accumulation of small integers). perf_mode=DoubleRow packs the
    full 256-channel contraction into one matmul per 3x3 tap.
  - Conv3x3 as implicit GEMM: per output tile [128 Cout x 392 cols] accumulate
    9 tap matmuls reading shifted windows of a zero-padded (30x30) binarized
    activation image.
  - BatchNorm uses full-batch statistics: per-channel sum/sumsq partials from
    bn_stats fused with PSUM evacuation, one tiny (2KB) AllGather per BN plus
    an on-device 8-way sum; a warm-up collective hides first-collective cost.
  - PReLU runs on the scalar engine's Prelu activation with runtime alpha.
"""

import numpy as np
import ml_dtypes

import concourse.bacc as bacc
import concourse.mybir as mybir
import concourse.tile as tile
from concourse.tile_rust import add_dep_helper
from concourse import bass_utils

N_CORES = 8
B_FULL, C, H, W = 64, 256, 28, 28
BL = B_FULL // N_CORES  # images per core
P = 128
NB = C // P             # channel blocks
HW = H * W              # 784
PADL = 30               # padded row length
PADQ = PADL * PADL      # 900 padded image
HALF = 14 * W           # 392 columns per psum tile (half an image)
NI_LOCAL = float(BL * HW)      # interior elems per core per channel
N_TOT = float(B_FULL * HW)     # full-batch elems per channel
SCALE = 0.1
BN_EPS = 1e-5

F32 = mybir.dt.float32
BF16 = mybir.dt.bfloat16
FP8 = mybir.dt.float8e4
BF16_NP = np.dtype(ml_dtypes.bfloat16)

_CACHE: dict = {}


def _build():
    nc = bacc.Bacc("TRN2", target_bir_lowering=False, debug=False,
                   num_devices=N_CORES)
    F = mybir.ActivationFunctionType
    Op = mybir.AluOpType
    DR = mybir.MatmulPerfMode.DoubleRow

    x_d = nc.dram_tensor("x", [BL, C, H, W], F32, kind="ExternalInput")
    xh_d = nc.dram_tensor("xh", [BL, C, H, W], BF16, kind="ExternalInput")
    # weights packed [ki, tap, i, mblk, co] with channel c = i*128 + ki
    w1_d = nc.dram_tensor("w1", [P, 9, NB, NB, P], FP8, kind="ExternalInput")
    w2_d = nc.dram_tensor("w2", [P, 9, NB, NB, P], FP8, kind="ExternalInput")
    # params packed [P, 6, NB]: order (s1,g1,be1,s2,g2,be2)
    par_d = nc.dram_tensor("par", [P, 6, NB], F32, kind="ExternalInput")
    a1_d = nc.dram_tensor("a1", [1], F32, kind="ExternalInput")
    a2_d = nc.dram_tensor("a2", [1], F32, kind="ExternalInput")
    o_d = nc.dram_tensor("o", [BL, C, H, W], F32, kind="ExternalOutput")

    with tile.TileContext(nc) as tc:
        with (
            tc.tile_pool(name="sbuf", bufs=1) as sbuf,
            tc.tile_pool(name="psum", bufs=6, space="PSUM") as psum_pool,
            tc.tile_pool(name="dram", bufs=1, space="DRAM") as dram,
        ):
            # ---- static parameters ----
            # conv1 weights load first: they gate the first matmul; conv2
            # weights aren't needed until halfway through the kernel
            w1_sb = sbuf.tile([P, 9, NB, NB, P], FP8)
            w2_sb = sbuf.tile([P, 9, NB, NB, P], FP8)
            nc.sync.dma_start(w1_sb[:], w1_d[:, :, :, :, :])

            par_sb = sbuf.tile([P, 6, NB], F32)
            nc.sync.dma_start(par_sb[:], par_d[:, :, :])
            s1_sb, g1_sb, be1_sb = (par_sb[:, i, :] for i in range(3))
            s2_sb, g2_sb, be2_sb = (par_sb[:, i, :] for i in range(3, 6))
            a1_sb = sbuf.tile([P, 1], F32)
            a2_sb = sbuf.tile([P, 1], F32)
            nc.sync.dma_start(a1_sb[:], a1_d[None, :].partition_broadcast(P))
            nc.sync.dma_start(a2_sb[:], a2_d[None, :].partition_broadcast(P))

            # s^2 and gamma*s (pre-collective, off the critical path)
            sq1 = sbuf.tile([P, NB], F32)
            gs1 = sbuf.tile([P, NB], F32)
            sq2 = sbuf.tile([P, NB], F32)
            gs2 = sbuf.tile([P, NB], F32)
            nc.vector.tensor_tensor(sq1[:], s1_sb, s1_sb, Op.mult)
            nc.vector.tensor_tensor(gs1[:], g1_sb, s1_sb, Op.mult)
            nc.vector.tensor_tensor(sq2[:], s2_sb, s2_sb, Op.mult)
            nc.vector.tensor_tensor(gs2[:], g2_sb, s2_sb, Op.mult)

            # tiny warm-up collective: absorbs the first-collective ncfw
            # wakeup / cross-core skew while conv1 runs
            warm_in = dram.tile([4, 2], F32, name="warm_in")
            warm_out = dram.tile([N_CORES * 4, 2], F32, name="warm_out",
                                 addr_space="Shared")
            nc.sync.dma_start(warm_in[:], par_d[0, 0:4, :])
            nc.gpsimd.collective_compute(
                "AllGather", Op.bypass,
                replica_groups=[list(range(N_CORES))],
                ins=[warm_in[:]], outs=[warm_out[:]])

            # ---- activations ----
            xb_img = [sbuf.tile([P, NB, PADQ], FP8, name=f"xb{b}")
                      for b in range(BL)]
            xh_img = [sbuf.tile([P, NB, HW], BF16, name=f"xh{b}")
                      for b in range(BL)]
            x_img = [sbuf.tile([P, NB, HW], F32, name=f"xr{b}")
                     for b in range(BL)]
            y_img = [sbuf.tile([P, NB, HW], F32, name=f"yy{b}")
                     for b in range(BL)]
            xbv = [t.rearrange("p k (r c) -> p k r c", c=PADL) for t in xb_img]
            xhv = [t.rearrange("p k (r c) -> p k r c", c=W) for t in xh_img]
            xv = [t.rearrange("p k (r c) -> p k r c", c=W) for t in x_img]
            yv = [t.rearrange("p k (r c) -> p k r c", c=W) for t in y_img]

            # bf16 copy of x loads fast (half the bytes, halves per-image
            # latency) and sign(bf16(x)) == sign(x); the f32 copy for the
            # residual streams in lazily under conv1
            xh_src = xh_d.rearrange("b (k p) h w -> b p k (h w)", p=P)
            last_xh_dma = None
            for b in range(BL):
                # zero only the pad borders; sign() fills the interior
                nc.vector.memset(xbv[b][:, :, 0:30:29, :], 0.0)
                nc.vector.memset(xbv[b][:, :, 1:29, 0:30:29], 0.0)
                for k in range(NB):
                    last_xh_dma = nc.sync.dma_start(xh_img[b][:, k, :],
                                                    xh_src[b, :, k, :])
                    nc.scalar.sign(xbv[b][:, k, 1:29, 1:29], xhv[b][:, k])
            # conv2 weights + f32 x (residual) stream in only after the bf16
            # copy is done so they don't steal DMA bandwidth from the conv1
            # critical path
            w2dma = nc.gpsimd.dma_start(w2_sb[:], w2_d[:, :, :, :, :])
            add_dep_helper(w2dma.ins, last_xh_dma.ins, sync=True,
                           reason="w2 load after bf16 x load")
            x_src = x_d.rearrange("b (k p) h w -> b p k (h w)", p=P)
            for b in range(BL):
                dma = nc.gpsimd.dma_start(x_img[b][:], x_src[b])
                add_dep_helper(dma.ins, last_xh_dma.ins, sync=True,
                               reason="f32 x load after bf16 x load")

            def conv(w_sb, y_out, st6):
                """bin-conv3x3 via DoubleRow fp8 (full 256-ch contraction per
                tap); writes raw integer conv sums + per-tile stats."""
                for b in range(BL):
                    for m in range(NB):
                        for hh in range(2):
                            ps = psum_pool.tile([P, HALF], F32, name="ps",
                                                tag="ps")
                            for t in range(9):
                                dh, dw = t // 3, t % 3
                                rhs = xbv[b][:, :,
                                             hh * 14 + dh:hh * 14 + dh + 14,
                                             dw:dw + 28]
                                nc.tensor.matmul(
                                    ps[:], w_sb[:, t, :, m, :], rhs,
                                    start=(t == 0), stop=(t == 8),
                                    perf_mode=DR)
                            dst = y_out[b][:, m, hh * HALF:(hh + 1) * HALF]
                            nc.scalar.copy(dst, ps[:])
                            idx = (b * 2 + hh) * 6
                            nc.vector.bn_stats(st6[:, m, idx:idx + 6], dst)

            def stats_to_ab(st6, sq_sb, gs_sb, be_sb, tagn):
                """aggregate local stats, AllGather per-channel [sum, sumsq]
                partials, 8-way sum, produce per-channel A,B."""
                st2 = sbuf.tile([P, NB, 2], F32, name=f"st2_{tagn}")
                for m in range(NB):
                    nc.vector.bn_aggr(st2[:, m], st6[:, m])
                cc_in = sbuf.tile([P, 4], F32, name=f"ccin_{tagn}")
                tmp = sbuf.tile([P, NB], F32, name=f"ctmp_{tagn}")
                # cols 0:2 = sum(y) per channel; 2:4 = sum(y^2)
                nc.vector.tensor_scalar(cc_in[:, 0:2], st2[:, :, 0], NI_LOCAL,
                                        None, Op.mult)
                nc.vector.tensor_tensor(tmp[:], st2[:, :, 0], st2[:, :, 0],
                                        Op.mult)
                nc.vector.tensor_tensor(tmp[:], st2[:, :, 1], tmp[:], Op.add)
                nc.vector.tensor_scalar(cc_in[:, 2:4], tmp[:], NI_LOCAL,
                                        None, Op.mult)
                cc_din = dram.tile([P, 4], F32, name=f"ccdin_{tagn}")
                cc_dout = dram.tile([N_CORES, P, 4], F32,
                                    name=f"ccdout_{tagn}",
                                    addr_space="Shared")
                nc.sync.dma_start(cc_din[:], cc_in[:])
                nc.gpsimd.collective_compute(
                    "AllGather", Op.bypass,
                    replica_groups=[list(range(N_CORES))],
                    ins=[cc_din[:]], outs=[cc_dout[:]])
                ccg = sbuf.tile([P, N_CORES, 4], F32, name=f"ccg_{tagn}")
                nc.sync.dma_start(ccg[:], cc_dout.rearrange("r p j -> p r j"))
                # 8-way tree sum -> [P, 4]
                nc.vector.tensor_tensor(ccg[:, 0:4], ccg[:, 0:4], ccg[:, 4:8],
                                        Op.add)
                nc.vector.tensor_tensor(ccg[:, 0:2], ccg[:, 0:2], ccg[:, 2:4],
                                        Op.add)
                cc_out = sbuf.tile([P, 4], F32, name=f"ccout_{tagn}")
                nc.vector.tensor_tensor(cc_out[:], ccg[:, 0], ccg[:, 1],
                                        Op.add)

                # [mean_m0, mean_m1, E[y^2]_m0, E[y^2]_m1] in one op
                mq = sbuf.tile([P, 4], F32, name=f"mq_{tagn}")
                nc.vector.tensor_scalar(mq[:], cc_out[:], 1.0 / N_TOT,
                                        None, Op.mult)
                mg = mq[:, 0:2]
                vg = sbuf.tile([P, NB], F32, name=f"vg_{tagn}")
                t0 = sbuf.tile([P, NB], F32, name=f"t0_{tagn}")
                d = sbuf.tile([P, NB], F32, name=f"d_{tagn}")
                r = sbuf.tile([P, NB], F32, name=f"r_{tagn}")
                av = sbuf.tile([P, NB], F32, name=f"av_{tagn}")
                bv = sbuf.tile([P, NB], F32, name=f"bv_{tagn}")
                nc.vector.tensor_tensor(t0[:], mg, mg, Op.mult)
                nc.vector.tensor_tensor(vg[:], mq[:, 2:4], t0[:], Op.subtract)
                # d = s^2 * var_y + eps   (== var(out) + eps up to rounding)
                nc.vector.tensor_tensor(d[:], sq_sb[:], vg[:], Op.mult)
                nc.vector.tensor_scalar(d[:], d[:], BN_EPS, None, Op.add)
                # r = rsqrt(d): sqrt+divide, then one Newton step
                nc.scalar.sqrt(t0[:], d[:])
                nc.vector.reciprocal(r[:], t0[:])
                nc.vector.tensor_tensor(t0[:], r[:], r[:], Op.mult)
                nc.vector.tensor_tensor(t0[:], t0[:], d[:], Op.mult)
                nc.vector.tensor_scalar(t0[:], t0[:], -0.5, 1.5, Op.mult,
                                        Op.add)
                nc.vector.tensor_tensor(r[:], r[:], t0[:], Op.mult)
                # A = gamma * s * r ; B = beta - mean_y * A
                nc.vector.tensor_tensor(av[:], gs_sb[:], r[:], Op.mult)
                nc.vector.tensor_tensor(t0[:], mg[:], av[:], Op.mult)
                nc.vector.tensor_tensor(bv[:], be_sb[:], t0[:], Op.subtract)
                return av, bv

            def post(src_imgs, res_imgs, av, bv, a_sb, write_xb2,
                     tt_gps=()):
                """per-image BN affine (DVE, out-of-place for the 2x perf
                mode; last image's on ACT) + residual add (DVE/GpSimd) +
                PReLU (ACT). Images in tt_gps do the residual on GpSimd."""
                for b in range(BL):
                    scr = sbuf.tile([P, NB, HW], F32, name=f"scr{b % 3}",
                                    tag="pscr", bufs=3)
                    for m in range(NB):
                        u = src_imgs[b][:, m, :]
                        if b == BL - 1:
                            nc.scalar.activation(scr[:, m, :], u, F.Identity,
                                                 bias=bv[:, m:m + 1],
                                                 scale=av[:, m:m + 1])
                        else:
                            nc.vector.tensor_scalar(scr[:, m, :], u,
                                                    av[:, m:m + 1],
                                                    bv[:, m:m + 1],
                                                    Op.mult, Op.add)
                    up = src_imgs[b][:, :, :]
                    tt_eng = nc.gpsimd if b in tt_gps else nc.vector
                    tt_eng.tensor_tensor(up, scr[:], res_imgs[b][:, :, :],
                                         Op.add)
                    nc.scalar.activation(
                        src_imgs[b].rearrange("p k i -> p (k i)"),
                        src_imgs[b].rearrange("p k i -> p (k i)"),
                        F.Prelu, bias=0.0, scale=1.0, alpha=a_sb[:, 0:1])
                    if write_xb2:
                        sv = src_imgs[b].rearrange("p k (r c) -> p k r c", c=W)
                        nc.scalar.sign(xbv[b][:, :, 1:29, 1:29], sv[:, :, :, :])

            # ================= stage 1 =================
            st6_1 = sbuf.tile([P, NB, BL * 12], F32)
            conv(w1_sb, y_img, st6_1)
            a1v, b1v = stats_to_ab(st6_1, sq1, gs1, be1_sb, "c1")
            post(y_img, x_img, a1v, b1v, a1_sb, write_xb2=True)

            # ================= stage 2 =================
            st6_2 = sbuf.tile([P, NB, BL * 12], F32)
            conv(w2_sb, x_img, st6_2)  # y2 overwrites x
            a2v, b2v = stats_to_ab(st6_2, sq2, gs2, be2_sb, "c2")
            post(x_img, y_img, a2v, b2v, a2_sb, write_xb2=False,
                 tt_gps=(5, 6, 7))

            o_dst = o_d.rearrange("b (k p) h w -> b p k (h w)", p=P)
            for b in range(BL):
                nc.sync.dma_start(o_dst[b], x_img[b][:])

    nc.compile()
    return nc


def _get_nc():
    if "nc" not in _CACHE:
        _CACHE["nc"] = _build()
    return _CACHE["nc"]


def _pack_w(w):
    wb = np.sign(np.asarray(w, np.float32))
    # [co, ci, kh, kw] -> [ki, tap, i, co_blk, co] with ci = i*128 + ki
    t = wb.reshape(NB, P, NB, P, 3, 3)
    t = np.transpose(t, (3, 4, 5, 2, 0, 1)).reshape(P, 9, NB, NB, P)
    return np.ascontiguousarray(t).astype(np.dtype(ml_dtypes.float8_e4m3))


def _pack_par(s1, g1, be1, s2, g2, be2):
    par = np.stack([np.asarray(v, np.float32).reshape(NB, P)
                    for v in (s1, g1, be1, s2, g2, be2)])  # [6, NB, P]
    return np.ascontiguousarray(par.transpose(2, 0, 1))    # [P, 6, NB]


def kernel(x, conv1_w, conv2_w, bn1_gamma, bn1_beta, bn2_gamma, bn2_beta,
           prelu1_a, prelu2_a):
    x = np.ascontiguousarray(np.asarray(x, np.float32))
    nc = _get_nc()

    s1 = SCALE * np.mean(np.abs(np.asarray(conv1_w, np.float32)),
                         axis=(1, 2, 3), dtype=np.float32)
    s2 = SCALE * np.mean(np.abs(np.asarray(conv2_w, np.float32)),
                         axis=(1, 2, 3), dtype=np.float32)

    shared = {
        "w1": _pack_w(conv1_w), "w2": _pack_w(conv2_w),
        "par": _pack_par(s1, bn1_gamma, bn1_beta, s2, bn2_gamma, bn2_beta),
        "a1": np.asarray(prelu1_a, np.float32).reshape(1),
        "a2": np.asarray(prelu2_a, np.float32).reshape(1),
    }
    xh = x.astype(BF16_NP)
    in_maps = [dict(shared, x=x[c * BL:(c + 1) * BL],
                    xh=xh[c * BL:(c + 1) * BL]) for c in range(N_CORES)]

    res = bass_utils.run_bass_kernel_spmd(nc, in_maps,
                                          core_ids=list(range(N_CORES)))
    out = np.concatenate([res.results[c]["o"] for c in range(N_CORES)], axis=0)
    return out
